# revision 37
# baseline (speedup 1.0000x reference)
"""Time-varying FIR (AllZeroDigitalFilter) on 8 TRN2 NeuronCores.

fp16 "C-decomposition", dual-engine (Vector + Scalar):
  C_k[i'] = sum_j h_pad[k,j] * x[(k-1)P + i' - j],  i' in [0,160)
  (filter of frame k applied across frames k-1 and k)
  y[kP+i] = w0[i]*C_k[80+i] + w1[i]*C_{k+1}[i]
This halves op count vs the direct A/B blend: one FD=160 op per tap
instead of two FD=80 ops. Per 126-row tile, N_DVE taps run as fused
mult-add chains on the Vector engine (scalar_tensor_tensor); the other
D-N_DVE tap-products run concurrently on the Scalar engine
(activation Copy with per-partition scale) into a contiguous 32-slot
product buffer, folded into the accumulator on Vector by a narrow
pre-fold (slots 16..N_ACT onto the front) plus an in-place 16-slot
halving tree — all wide fp16 2x-mode adds, ~2.3us per tile. The cross-partition (+1) combine uses a
partition-shifted SBUF->SBUF DMA + one tensor add; output is staged in
an fp16 DRAM buffer and cast to fp32 with one SWDGE cast-DMA pass.
Precision (validated vs reference): ~7.9e-4 relative error.
Sharding: pure data parallel across batch, 2 sequences per core.

Sync design note: cumulative thresholds on a shared DMA semaphore are
unsound with >1 DMA in flight (per-SDMA-engine completion skew lets a
later tile's increments satisfy an earlier tile's threshold). Buffer-
parity semaphores make every threshold equal to the maximum possible
increment count at wait time, so a fired wait implies full completion.
"""

import sys

for p in ("/opt/trn_rl_repo", "/root/.axon_site/_ro/trn_rl_repo"):
    if p not in sys.path:
        sys.path.append(p)

import numpy as np
import concourse.bass as bass
import concourse.mybir as mybir
from concourse.ap import AP
from concourse.bass_utils import run_bass_kernel_spmd

B, T = 16, 80000
P, D = 80, 50  # frame period, taps
N = T // P  # 1000 frames
W2 = 2 * P + D - 1  # 209: extended window for the 160-wide C rows
NCORES = 8
S = B // NCORES  # sequences per core
FO = 125  # output frames per tile
FT = FO + 1  # C-rows per tile (tiles overlap by 1 row)
NTSEQ = N // FO  # 8 tiles per sequence
PAD = D - 1 + P  # front pad so C_k window starts at k*P: 129
TPC = N * P + W2 + 2  # padded x length (+2 slack for the odd-offset copy)

F16 = mybir.dt.float16
FP32 = mybir.dt.float32

N_DVE = 25  # taps computed on the Vector engine (fused mult-add chain)
# remaining D - N_DVE taps: products on the Scalar (ACT) engine, folded in
# with fp16 2x-mode tensor adds on DVE

_nc_cache = {}


def build_nc():
    if "nc" in _nc_cache:
        return _nc_cache["nc"]
    nc = bass.Bass()
    xp_ext = nc.declare_dram_parameter("xp", [S, TPC], F16, isOutput=False)
    hc_ext = nc.declare_dram_parameter("hc", [S, N + 1, D], FP32, isOutput=False)
    rr_ext = nc.declare_dram_parameter("rr", [128, 2 * P], F16, isOutput=False)
    out_ext = nc.declare_dram_parameter("out", [S, T], FP32, isOutput=True)
    ydram = nc.dram_tensor("ydram", [S, T], F16)

    NTILES = S * NTSEQ  # tile t -> seq s = t // NTSEQ, chunk ci = t % NTSEQ

    from contextlib import ExitStack

    with ExitStack() as _ctx:
        ec = _ctx.enter_context
        xa0 = ec(nc.sbuf_tensor([FT, W2], F16))
        xa1 = ec(nc.sbuf_tensor([FT, W2], F16))
        xb0 = ec(nc.sbuf_tensor([FT, W2], F16))
        xb1 = ec(nc.sbuf_tensor([FT, W2], F16))
        hh0 = ec(nc.sbuf_tensor([FT, D], FP32))
        hh1 = ec(nc.sbuf_tensor([FT, D], FP32))
        acc0 = ec(nc.sbuf_tensor([FT, 2 * P], F16))
        acc1 = ec(nc.sbuf_tensor([FT, 2 * P], F16))
        vt = ec(nc.sbuf_tensor([FT, 2 * P], F16))
        vs = ec(nc.sbuf_tensor([FO, P], F16))
        y0 = ec(nc.sbuf_tensor([FO, P], F16))
        y1 = ec(nc.sbuf_tensor([FO, P], F16))
        rrt = ec(nc.sbuf_tensor([128, 2 * P], F16))
        ramp_sem = ec(nc.semaphore("ramp_sem"))
        dma_e = ec(nc.semaphore("dma_e"))
        dma_o = ec(nc.semaphore("dma_o"))
        v_sem = ec(nc.semaphore("v_sem"))
        vs_sem = ec(nc.semaphore("vs_sem"))
        ya_sem = ec(nc.semaphore("ya_sem"))
        out_e = ec(nc.semaphore("out_e"))
        out_o = ec(nc.semaphore("out_o"))
        cast_sem = ec(nc.semaphore("cast_sem"))
        act_sem = ec(nc.semaphore("act_sem"))
        N_ACT = D - N_DVE
        # all 50 tap-products (both engines) land in one 50-slot buffer;
        # 50 = 32 + 18, so a narrow pre-fold + 32-slot halving tree needs
        # no zero padding at all
        prb0 = ec(nc.sbuf_tensor([FT, D * 2 * P], F16))
        prb1 = ec(nc.sbuf_tensor([FT, D * 2 * P], F16))
        prb = [prb0, prb1]
        block = ec(nc.Block())
        xa = [xa0, xa1]
        xb = [xb0, xb1]
        hh = [hh0, hh1]
        yt = [y0, y1]
        dma_s = [dma_e, dma_o]
        out_s = [out_e, out_o]

        def ydst(t):
            s, ci = t // NTSEQ, t % NTSEQ
            return AP(
                tensor=ydram[:].tensor,
                offset=s * T + ci * FO * P,
                ap=[[P, FO], [1, P]],
            )

        @block.sync
        def _(sync):
            for t in range(NTILES):
                s, ci = t // NTSEQ, t % NTSEQ
                b = t % 2
                k0 = ci * FO
                if t >= 2:
                    sync.wait_ge(v_sem, t - 1)  # WAR: tile t-2 read its inputs
                src_a = AP(
                    tensor=xp_ext[:].tensor,
                    offset=s * TPC + k0 * P,
                    ap=[[P, FT], [1, W2]],
                )
                src_b = AP(
                    tensor=xp_ext[:].tensor,
                    offset=s * TPC + k0 * P + 1,
                    ap=[[P, FT], [1, W2]],
                )
                sync.dma_start(xa[b][:], src_a).then_inc(dma_s[b], 16)
                sync.dma_start(xb[b][:], src_b).then_inc(dma_s[b], 16)
                sync.dma_start(hh[b][:], hc_ext[s, k0 : k0 + FT, :]).then_inc(
                    dma_s[b], 16
                )
                if t == 0:
                    sync.dma_start(rrt[:], rr_ext[:]).then_inc(ramp_sem, 16)
                if t >= 1:
                    # partition-shift copy of V rows 1..FT for tile t-1
                    sync.wait_ge(v_sem, t)
                    sync.dma_start(vs[:], vt[1:FT, 0:P]).then_inc(vs_sem, 16)
                if t >= 2:
                    # store y of tile t-2 (ya available early; avoids blocking
                    # the next tile's input DMAs behind tile t-1's compute)
                    sync.wait_ge(ya_sem, t - 1)
                    sync.dma_start(ydst(t - 2), yt[(t - 2) % 2][:]).then_inc(
                        out_s[(t - 2) % 2], 16
                    )
            # tail: last tile's shift + remaining stores
            tl = NTILES - 1
            sync.wait_ge(v_sem, NTILES)
            sync.dma_start(vs[:], vt[1:FT, 0:P]).then_inc(vs_sem, 16)
            sync.wait_ge(ya_sem, NTILES - 1)
            sync.dma_start(ydst(tl - 1), yt[(tl - 1) % 2][:]).then_inc(
                out_s[(tl - 1) % 2], 16
            )
            sync.wait_ge(ya_sem, NTILES)
            sync.dma_start(ydst(tl), yt[tl % 2][:]).then_inc(out_s[tl % 2], 16)
            sync.wait_ge(out_s[tl % 2], 16 * (tl // 2 + 1))
            sync.wait_ge(out_s[1 - tl % 2], 16 * ((tl - 1) // 2 + 1))
            sync.sem_inc(cast_sem, 1)
            # after the gpsimd cast pass completes, kernel may end
            sync.wait_ge(cast_sem, 17)

        def src_for(buf_pair, b, j):
            # slice of the extended window for tap j, 4B-aligned via the
            # one-element-shifted copy when the natural offset is odd
            off = D - 1 - j
            if off % 2 == 0:
                return buf_pair[0][b][:, off : off + 2 * P]
            return buf_pair[1][b][:, off - 1 : off - 1 + 2 * P]

        @block.vector
        def _(vector):
            def conv(t):
                b = t % 2
                vector.wait_ge(dma_s[b], 48 * (t // 2 + 1))
                # DVE's own tap-products (tensor_scalar hits the 4x perf mode;
                # cheaper per tap than the fused mult-add chain)
                for j in range(N_DVE):
                    vector.tensor_scalar_mul(
                        prb[b][:, j * 2 * P : (j + 1) * 2 * P],
                        src_for((xa, xb), b, j),
                        hh[b][:, j : j + 1],
                    )
                # fold all 50 products: narrow pre-fold 50 -> 32, then an
                # in-place 32-slot halving tree (50 = 32+18: no padding)
                vector.wait_ge(act_sem, t + 1)
                extra = D - 32
                vector.tensor_tensor(
                    out=prb[b][:, 0 : extra * 2 * P],
                    in0=prb[b][:, 0 : extra * 2 * P],
                    in1=prb[b][:, 32 * 2 * P : D * 2 * P],
                    op=mybir.AluOpType.add,
                )
                width = 32 * 2 * P
                while width > 2 * P:
                    half = width // 2
                    vector.tensor_tensor(
                        out=prb[b][:, 0:half],
                        in0=prb[b][:, 0:half],
                        in1=prb[b][:, half:width],
                        op=mybir.AluOpType.add,
                    )
                    width = half
                return prb[b][:, 0 : 2 * P]

            for t in range(NTILES):
                fin = conv(t)
                if t == 0:
                    vector.wait_ge(ramp_sem, 16)
                if t >= 1:
                    # combine tile t-1: y = V[0:FO, 80:160] + Vs
                    vector.wait_ge(vs_sem, 16 * t)
                    if t - 1 >= 2:
                        vector.wait_ge(out_s[(t - 1) % 2], 16 * ((t - 1) // 2))
                    vector.tensor_tensor(
                        out=yt[(t - 1) % 2][:],
                        in0=vt[0:FO, P : 2 * P],
                        in1=vs[:],
                        op=mybir.AluOpType.add,
                    ).then_inc(ya_sem, 1)
                # V_t = C_t * rr
                vector.tensor_tensor(
                    out=vt[:], in0=fin[:], in1=rrt[0:FT, :], op=mybir.AluOpType.mult
                ).then_inc(v_sem, 1)
            # tail combine for last tile
            tl = NTILES - 1
            vector.wait_ge(vs_sem, 16 * NTILES)
            vector.wait_ge(out_s[tl % 2], 16 * (tl // 2))
            vector.tensor_tensor(
                out=yt[tl % 2][:],
                in0=vt[0:FO, P : 2 * P],
                in1=vs[:],
                op=mybir.AluOpType.add,
            ).then_inc(ya_sem, 1)

        @block.scalar
        def _(scalar):
            for t in range(NTILES):
                b = t % 2
                scalar.wait_ge(dma_s[b], 48 * (t // 2 + 1))
                if t >= 2:
                    scalar.wait_ge(v_sem, t - 1)  # WAR on pr[b] scratch
                for idx, j in enumerate(range(N_DVE, D)):
                    inst = scalar.activation(
                        prb[b][:, j * 2 * P : (j + 1) * 2 * P],
                        src_for((xa, xb), b, j),
                        mybir.ActivationFunctionType.Copy,
                        scale=hh[b][:, j : j + 1],
                    )
                    if idx == N_ACT - 1:
                        inst.then_inc(act_sem, 1)

        @block.gpsimd
        def _(gpsimd):
            # final cast pass fp16 -> fp32 (SWDGE dtype-cast DMA)
            gpsimd.wait_ge(cast_sem, 1)
            gpsimd.dma_start(out_ext[:], ydram[:]).then_inc(cast_sem, 16)

    _nc_cache["nc"] = nc
    return nc


def _prep_core_inputs(x, h):
    x = np.ascontiguousarray(x, dtype=np.float32)
    h = np.ascontiguousarray(h, dtype=np.float32)
    xp = np.zeros((B, TPC), np.float16)
    xp[:, PAD : PAD + T] = x.astype(np.float16)
    hpad = np.ascontiguousarray(np.concatenate([h, h[:, -1:, :]], axis=1))  # (B,N+1,D) f32
    w1 = (np.arange(P, dtype=np.float32) / P).astype(np.float16)
    w0 = (1.0 - np.arange(P, dtype=np.float32) / P).astype(np.float16)
    rr = np.broadcast_to(
        np.concatenate([w1, w0])[None, :], (128, 2 * P)
    )
    rr = np.ascontiguousarray(rr)
    in_maps = []
    for c in range(NCORES):
        sl = slice(c * S, (c + 1) * S)
        in_maps.append({"xp": xp[sl], "hc": hpad[sl], "rr": rr})
    return in_maps


def kernel(x, h, **kw):
    nc = build_nc()
    in_maps = _prep_core_inputs(x, h)
    res = run_bass_kernel_spmd(nc, in_maps, core_ids=list(range(NCORES)), **kw)
    out = np.concatenate([res.results[c]["out"] for c in range(NCORES)], axis=0)
    return np.ascontiguousarray(out, dtype=np.float32)


def kernel_traced(x, h, **kw):
    nc = build_nc()
    in_maps = _prep_core_inputs(x, h)
    res = run_bass_kernel_spmd(
        nc, in_maps, core_ids=list(range(NCORES)), trace=True, **kw
    )
    out = np.concatenate([res.results[c]["out"] for c in range(NCORES)], axis=0)
    return np.ascontiguousarray(out, dtype=np.float32), res


# revision 38
# speedup vs baseline: 1.0285x; 1.0285x over previous
"""Time-varying FIR (AllZeroDigitalFilter) on 8 TRN2 NeuronCores.

fp16 "C-decomposition", dual-engine (Vector + Scalar):
  C_k[i'] = sum_j h_pad[k,j] * x[(k-1)P + i' - j],  i' in [0,160)
  (filter of frame k applied across frames k-1 and k)
  y[kP+i] = w0[i]*C_k[80+i] + w1[i]*C_{k+1}[i]
This halves op count vs the direct A/B blend: one FD=160 op per tap
instead of two FD=80 ops. Per 126-row tile, N_DVE taps run as fused
mult-add chains on the Vector engine (scalar_tensor_tensor); the other
D-N_DVE tap-products run concurrently on the Scalar engine
(activation Copy with per-partition scale) into a contiguous 32-slot
product buffer, folded into the accumulator on Vector by a narrow
pre-fold (slots 16..N_ACT onto the front) plus an in-place 16-slot
halving tree — all wide fp16 2x-mode adds, ~2.3us per tile. The cross-partition (+1) combine uses a
partition-shifted SBUF->SBUF DMA + one tensor add; output is staged in
an fp16 DRAM buffer and cast to fp32 with one SWDGE cast-DMA pass.
Precision (validated vs reference): ~7.9e-4 relative error.
Sharding: pure data parallel across batch, 2 sequences per core.

Sync design note: cumulative thresholds on a shared DMA semaphore are
unsound with >1 DMA in flight (per-SDMA-engine completion skew lets a
later tile's increments satisfy an earlier tile's threshold). Buffer-
parity semaphores make every threshold equal to the maximum possible
increment count at wait time, so a fired wait implies full completion.
"""

import sys

for p in ("/opt/trn_rl_repo", "/root/.axon_site/_ro/trn_rl_repo"):
    if p not in sys.path:
        sys.path.append(p)

import numpy as np
import concourse.bass as bass
import concourse.mybir as mybir
from concourse.ap import AP
from concourse.bass_utils import run_bass_kernel_spmd

B, T = 16, 80000
P, D = 80, 50  # frame period, taps
N = T // P  # 1000 frames
W2 = 2 * P + D - 1  # 209: extended window for the 160-wide C rows
NCORES = 8
S = B // NCORES  # sequences per core
FO = 125  # output frames per tile
FT = FO + 1  # C-rows per tile (tiles overlap by 1 row)
NTSEQ = N // FO  # 8 tiles per sequence
PAD = D - 1 + P  # front pad so C_k window starts at k*P: 129
TPC = N * P + W2 + 2  # padded x length (+2 slack for the odd-offset copy)

F16 = mybir.dt.float16
FP32 = mybir.dt.float32

N_DVE = 25  # taps computed on the Vector engine (fused mult-add chain)
# remaining D - N_DVE taps: products on the Scalar (ACT) engine, folded in
# with fp16 2x-mode tensor adds on DVE

_nc_cache = {}


def build_nc():
    if "nc" in _nc_cache:
        return _nc_cache["nc"]
    nc = bass.Bass()
    xp_ext = nc.declare_dram_parameter("xp", [S, TPC], F16, isOutput=False)
    hc_ext = nc.declare_dram_parameter("hc", [S, N + 1, D], FP32, isOutput=False)
    rr_ext = nc.declare_dram_parameter("rr", [128, 2 * P], F16, isOutput=False)
    out_ext = nc.declare_dram_parameter("out", [S, T], FP32, isOutput=True)
    ydram = nc.dram_tensor("ydram", [S, T], F16)

    NTILES = S * NTSEQ  # tile t -> seq s = t // NTSEQ, chunk ci = t % NTSEQ

    from contextlib import ExitStack

    with ExitStack() as _ctx:
        ec = _ctx.enter_context
        xa0 = ec(nc.sbuf_tensor([FT, W2], F16))
        xa1 = ec(nc.sbuf_tensor([FT, W2], F16))
        xb0 = ec(nc.sbuf_tensor([FT, W2], F16))
        xb1 = ec(nc.sbuf_tensor([FT, W2], F16))
        hh0 = ec(nc.sbuf_tensor([FT, D], FP32))
        hh1 = ec(nc.sbuf_tensor([FT, D], FP32))
        acc0 = ec(nc.sbuf_tensor([FT, 2 * P], F16))
        acc1 = ec(nc.sbuf_tensor([FT, 2 * P], F16))
        vt = ec(nc.sbuf_tensor([FT, 2 * P], F16))
        vs = ec(nc.sbuf_tensor([FO, P], F16))
        y0 = ec(nc.sbuf_tensor([FO, P], F16))
        y1 = ec(nc.sbuf_tensor([FO, P], F16))
        rrt = ec(nc.sbuf_tensor([128, 2 * P], F16))
        ramp_sem = ec(nc.semaphore("ramp_sem"))
        dma_e = ec(nc.semaphore("dma_e"))
        dma_o = ec(nc.semaphore("dma_o"))
        v_sem = ec(nc.semaphore("v_sem"))
        vs_sem = ec(nc.semaphore("vs_sem"))
        ya_sem = ec(nc.semaphore("ya_sem"))
        out_e = ec(nc.semaphore("out_e"))
        out_o = ec(nc.semaphore("out_o"))
        cast_sem = ec(nc.semaphore("cast_sem"))
        act_sem = ec(nc.semaphore("act_sem"))
        N_ACT = D - N_DVE
        NSLOT = 32  # padded to a power of two for the in-place halving tree
        assert N_ACT <= NSLOT
        prb0 = ec(nc.sbuf_tensor([FT, NSLOT * 2 * P], F16))
        prb1 = ec(nc.sbuf_tensor([FT, NSLOT * 2 * P], F16))
        prb = [prb0, prb1]
        block = ec(nc.Block())
        xa = [xa0, xa1]
        xb = [xb0, xb1]
        hh = [hh0, hh1]
        yt = [y0, y1]
        dma_s = [dma_e, dma_o]
        out_s = [out_e, out_o]

        def ydst(t):
            s, ci = t // NTSEQ, t % NTSEQ
            return AP(
                tensor=ydram[:].tensor,
                offset=s * T + ci * FO * P,
                ap=[[P, FO], [1, P]],
            )

        @block.sync
        def _(sync):
            for t in range(NTILES):
                s, ci = t // NTSEQ, t % NTSEQ
                b = t % 2
                k0 = ci * FO
                if t >= 2:
                    sync.wait_ge(v_sem, t - 1)  # WAR: tile t-2 read its inputs
                src_a = AP(
                    tensor=xp_ext[:].tensor,
                    offset=s * TPC + k0 * P,
                    ap=[[P, FT], [1, W2]],
                )
                src_b = AP(
                    tensor=xp_ext[:].tensor,
                    offset=s * TPC + k0 * P + 1,
                    ap=[[P, FT], [1, W2]],
                )
                sync.dma_start(xa[b][:], src_a).then_inc(dma_s[b], 16)
                sync.dma_start(xb[b][:], src_b).then_inc(dma_s[b], 16)
                sync.dma_start(hh[b][:], hc_ext[s, k0 : k0 + FT, :]).then_inc(
                    dma_s[b], 16
                )
                if t == 0:
                    sync.dma_start(rrt[:], rr_ext[:]).then_inc(ramp_sem, 16)
                if t >= 1:
                    # partition-shift copy of V rows 1..FT for tile t-1
                    sync.wait_ge(v_sem, t)
                    sync.dma_start(vs[:], vt[1:FT, 0:P]).then_inc(vs_sem, 16)
                if t >= 2:
                    # store y of tile t-2 (ya available early; avoids blocking
                    # the next tile's input DMAs behind tile t-1's compute)
                    sync.wait_ge(ya_sem, t - 1)
                    sync.dma_start(ydst(t - 2), yt[(t - 2) % 2][:]).then_inc(
                        out_s[(t - 2) % 2], 16
                    )
            # tail: last tile's shift + remaining stores
            tl = NTILES - 1
            sync.wait_ge(v_sem, NTILES)
            sync.dma_start(vs[:], vt[1:FT, 0:P]).then_inc(vs_sem, 16)
            sync.wait_ge(ya_sem, NTILES - 1)
            sync.dma_start(ydst(tl - 1), yt[(tl - 1) % 2][:]).then_inc(
                out_s[(tl - 1) % 2], 16
            )
            sync.wait_ge(ya_sem, NTILES)
            sync.dma_start(ydst(tl), yt[tl % 2][:]).then_inc(out_s[tl % 2], 16)
            sync.wait_ge(out_s[tl % 2], 16 * (tl // 2 + 1))
            sync.wait_ge(out_s[1 - tl % 2], 16 * ((tl - 1) // 2 + 1))
            sync.sem_inc(cast_sem, 1)
            # after the gpsimd cast pass completes, kernel may end
            sync.wait_ge(cast_sem, 17)

        def src_for(buf_pair, b, j):
            # slice of the extended window for tap j, 4B-aligned via the
            # one-element-shifted copy when the natural offset is odd
            off = D - 1 - j
            if off % 2 == 0:
                return buf_pair[0][b][:, off : off + 2 * P]
            return buf_pair[1][b][:, off - 1 : off - 1 + 2 * P]

        @block.vector
        def _(vector):
            def conv(t):
                b = t % 2
                accs = [acc0, acc1]
                vector.wait_ge(dma_s[b], 48 * (t // 2 + 1))
                vector.tensor_scalar_mul(acc0[:], src_for((xa, xb), b, 0), hh[b][:, 0:1])
                cur = 0
                for j in range(1, N_DVE):
                    nxt = 1 - cur
                    vector.scalar_tensor_tensor(
                        out=accs[nxt][:],
                        in0=src_for((xa, xb), b, j),
                        scalar=hh[b][:, j : j + 1],
                        in1=accs[cur][:],
                        op0=mybir.AluOpType.mult,
                        op1=mybir.AluOpType.add,
                    )
                    cur = nxt
                # fold in the ACT-engine products: narrow pre-level folds the
                # slots beyond 16 onto the front (no zero padding needed), then
                # an in-place halving tree over the remaining 16 slots
                vector.wait_ge(act_sem, t + 1)
                if N_ACT > 16:
                    extra = N_ACT - 16
                    vector.tensor_tensor(
                        out=prb[b][:, 0 : extra * 2 * P],
                        in0=prb[b][:, 0 : extra * 2 * P],
                        in1=prb[b][:, 16 * 2 * P : N_ACT * 2 * P],
                        op=mybir.AluOpType.add,
                    )
                    width = 16 * 2 * P
                else:
                    width = NSLOT * 2 * P
                while width > 2 * P:
                    half = width // 2
                    vector.tensor_tensor(
                        out=prb[b][:, 0:half],
                        in0=prb[b][:, 0:half],
                        in1=prb[b][:, half:width],
                        op=mybir.AluOpType.add,
                    )
                    width = half
                nxt = 1 - cur
                vector.tensor_tensor(
                    out=accs[nxt][:],
                    in0=accs[cur][:],
                    in1=prb[b][:, 0 : 2 * P],
                    op=mybir.AluOpType.add,
                )
                cur = nxt
                return accs[cur]

            if N_ACT <= 16:
                # one-time zeroing of padding slots for the pure halving tree
                for pp in range(2):
                    vector.memset(prb[pp][:, N_ACT * 2 * P : NSLOT * 2 * P], 0.0)
            for t in range(NTILES):
                fin = conv(t)
                if t == 0:
                    vector.wait_ge(ramp_sem, 16)
                if t >= 1:
                    # combine tile t-1: y = V[0:FO, 80:160] + Vs
                    vector.wait_ge(vs_sem, 16 * t)
                    if t - 1 >= 2:
                        vector.wait_ge(out_s[(t - 1) % 2], 16 * ((t - 1) // 2))
                    vector.tensor_tensor(
                        out=yt[(t - 1) % 2][:],
                        in0=vt[0:FO, P : 2 * P],
                        in1=vs[:],
                        op=mybir.AluOpType.add,
                    ).then_inc(ya_sem, 1)
                # V_t = C_t * rr
                vector.tensor_tensor(
                    out=vt[:], in0=fin[:], in1=rrt[0:FT, :], op=mybir.AluOpType.mult
                ).then_inc(v_sem, 1)
            # tail combine for last tile
            tl = NTILES - 1
            vector.wait_ge(vs_sem, 16 * NTILES)
            vector.wait_ge(out_s[tl % 2], 16 * (tl // 2))
            vector.tensor_tensor(
                out=yt[tl % 2][:],
                in0=vt[0:FO, P : 2 * P],
                in1=vs[:],
                op=mybir.AluOpType.add,
            ).then_inc(ya_sem, 1)

        @block.scalar
        def _(scalar):
            for t in range(NTILES):
                b = t % 2
                scalar.wait_ge(dma_s[b], 48 * (t // 2 + 1))
                if t >= 2:
                    scalar.wait_ge(v_sem, t - 1)  # WAR on pr[b] scratch
                for idx, j in enumerate(range(N_DVE, D)):
                    inst = scalar.activation(
                        prb[b][:, idx * 2 * P : (idx + 1) * 2 * P],
                        src_for((xa, xb), b, j),
                        mybir.ActivationFunctionType.Copy,
                        scale=hh[b][:, j : j + 1],
                    )
                    if idx == N_ACT - 1:
                        inst.then_inc(act_sem, 1)

        @block.gpsimd
        def _(gpsimd):
            # final cast pass fp16 -> fp32 (SWDGE dtype-cast DMA)
            gpsimd.wait_ge(cast_sem, 1)
            gpsimd.dma_start(out_ext[:], ydram[:]).then_inc(cast_sem, 16)

    _nc_cache["nc"] = nc
    return nc


def _prep_core_inputs(x, h):
    x = np.ascontiguousarray(x, dtype=np.float32)
    h = np.ascontiguousarray(h, dtype=np.float32)
    xp = np.zeros((B, TPC), np.float16)
    xp[:, PAD : PAD + T] = x.astype(np.float16)
    hpad = np.ascontiguousarray(np.concatenate([h, h[:, -1:, :]], axis=1))  # (B,N+1,D) f32
    w1 = (np.arange(P, dtype=np.float32) / P).astype(np.float16)
    w0 = (1.0 - np.arange(P, dtype=np.float32) / P).astype(np.float16)
    rr = np.broadcast_to(
        np.concatenate([w1, w0])[None, :], (128, 2 * P)
    )
    rr = np.ascontiguousarray(rr)
    in_maps = []
    for c in range(NCORES):
        sl = slice(c * S, (c + 1) * S)
        in_maps.append({"xp": xp[sl], "hc": hpad[sl], "rr": rr})
    return in_maps


def kernel(x, h, **kw):
    nc = build_nc()
    in_maps = _prep_core_inputs(x, h)
    res = run_bass_kernel_spmd(nc, in_maps, core_ids=list(range(NCORES)), **kw)
    out = np.concatenate([res.results[c]["out"] for c in range(NCORES)], axis=0)
    return np.ascontiguousarray(out, dtype=np.float32)


def kernel_traced(x, h, **kw):
    nc = build_nc()
    in_maps = _prep_core_inputs(x, h)
    res = run_bass_kernel_spmd(
        nc, in_maps, core_ids=list(range(NCORES)), trace=True, **kw
    )
    out = np.concatenate([res.results[c]["out"] for c in range(NCORES)], axis=0)
    return np.ascontiguousarray(out, dtype=np.float32), res


# revision 39
# speedup vs baseline: 1.0358x; 1.0072x over previous
"""Time-varying FIR (AllZeroDigitalFilter) on 8 TRN2 NeuronCores.

fp16 "C-decomposition", dual-engine (Vector + Scalar):
  C_k[i'] = sum_j h_pad[k,j] * x[(k-1)P + i' - j],  i' in [0,160)
  (filter of frame k applied across frames k-1 and k)
  y[kP+i] = w0[i]*C_k[80+i] + w1[i]*C_{k+1}[i]
This halves op count vs the direct A/B blend: one FD=160 op per tap
instead of two FD=80 ops. Per 126-row tile, N_DVE taps run as fused
mult-add chains on the Vector engine (scalar_tensor_tensor); the other
D-N_DVE tap-products run concurrently on the Scalar engine
(activation Copy with per-partition scale) into a contiguous 32-slot
product buffer, folded into the accumulator on Vector by a narrow
pre-fold (slots 16..N_ACT onto the front) plus an in-place 16-slot
halving tree — all wide fp16 2x-mode adds, ~2.3us per tile. The cross-partition (+1) combine uses a
partition-shifted SBUF->SBUF DMA + one tensor add; output is staged in
an fp16 DRAM buffer and cast to fp32 with one SWDGE cast-DMA pass.
Precision (validated vs reference): ~7.9e-4 relative error.
Sharding: pure data parallel across batch, 2 sequences per core.

Sync design note: cumulative thresholds on a shared DMA semaphore are
unsound with >1 DMA in flight (per-SDMA-engine completion skew lets a
later tile's increments satisfy an earlier tile's threshold). Buffer-
parity semaphores make every threshold equal to the maximum possible
increment count at wait time, so a fired wait implies full completion.
"""

import sys

for p in ("/opt/trn_rl_repo", "/root/.axon_site/_ro/trn_rl_repo"):
    if p not in sys.path:
        sys.path.append(p)

import numpy as np
import concourse.bass as bass
import concourse.mybir as mybir
from concourse.ap import AP
from concourse.bass_utils import run_bass_kernel_spmd

B, T = 16, 80000
P, D = 80, 50  # frame period, taps
N = T // P  # 1000 frames
W2 = 2 * P + D - 1  # 209: extended window for the 160-wide C rows
NCORES = 8
S = B // NCORES  # sequences per core
FO = 125  # output frames per tile
FT = FO + 1  # C-rows per tile (tiles overlap by 1 row)
NTSEQ = N // FO  # 8 tiles per sequence
PAD = D - 1 + P  # front pad so C_k window starts at k*P: 129
TPC = N * P + W2 + 2  # padded x length (+2 slack for the odd-offset copy)

F16 = mybir.dt.float16
FP32 = mybir.dt.float32

N_DVE = 26  # taps computed on the Vector engine (fused mult-add chain)
# remaining D - N_DVE taps: products on the Scalar (ACT) engine, folded in
# with fp16 2x-mode tensor adds on DVE

_nc_cache = {}


def build_nc():
    if "nc" in _nc_cache:
        return _nc_cache["nc"]
    nc = bass.Bass()
    xp_ext = nc.declare_dram_parameter("xp", [S, TPC], F16, isOutput=False)
    hc_ext = nc.declare_dram_parameter("hc", [S, N + 1, D], FP32, isOutput=False)
    rr_ext = nc.declare_dram_parameter("rr", [128, 2 * P], F16, isOutput=False)
    out_ext = nc.declare_dram_parameter("out", [S, T], FP32, isOutput=True)
    ydram = nc.dram_tensor("ydram", [S, T], F16)

    NTILES = S * NTSEQ  # tile t -> seq s = t // NTSEQ, chunk ci = t % NTSEQ

    from contextlib import ExitStack

    with ExitStack() as _ctx:
        ec = _ctx.enter_context
        xa0 = ec(nc.sbuf_tensor([FT, W2], F16))
        xa1 = ec(nc.sbuf_tensor([FT, W2], F16))
        xb0 = ec(nc.sbuf_tensor([FT, W2], F16))
        xb1 = ec(nc.sbuf_tensor([FT, W2], F16))
        hh0 = ec(nc.sbuf_tensor([FT, D], FP32))
        hh1 = ec(nc.sbuf_tensor([FT, D], FP32))
        acc0 = ec(nc.sbuf_tensor([FT, 2 * P], F16))
        acc1 = ec(nc.sbuf_tensor([FT, 2 * P], F16))
        vt = ec(nc.sbuf_tensor([FT, 2 * P], F16))
        vs = ec(nc.sbuf_tensor([FO, P], F16))
        y0 = ec(nc.sbuf_tensor([FO, P], F16))
        y1 = ec(nc.sbuf_tensor([FO, P], F16))
        rrt = ec(nc.sbuf_tensor([128, 2 * P], F16))
        ramp_sem = ec(nc.semaphore("ramp_sem"))
        dma_e = ec(nc.semaphore("dma_e"))
        dma_o = ec(nc.semaphore("dma_o"))
        v_sem = ec(nc.semaphore("v_sem"))
        vs_sem = ec(nc.semaphore("vs_sem"))
        ya_sem = ec(nc.semaphore("ya_sem"))
        out_e = ec(nc.semaphore("out_e"))
        out_o = ec(nc.semaphore("out_o"))
        cast_sem = ec(nc.semaphore("cast_sem"))
        act_sem = ec(nc.semaphore("act_sem"))
        N_ACT = D - N_DVE
        NSLOT = 32  # padded to a power of two for the in-place halving tree
        assert N_ACT <= NSLOT
        prb0 = ec(nc.sbuf_tensor([FT, NSLOT * 2 * P], F16))
        prb1 = ec(nc.sbuf_tensor([FT, NSLOT * 2 * P], F16))
        prb = [prb0, prb1]
        block = ec(nc.Block())
        xa = [xa0, xa1]
        xb = [xb0, xb1]
        hh = [hh0, hh1]
        yt = [y0, y1]
        dma_s = [dma_e, dma_o]
        out_s = [out_e, out_o]

        def ydst(t):
            s, ci = t // NTSEQ, t % NTSEQ
            return AP(
                tensor=ydram[:].tensor,
                offset=s * T + ci * FO * P,
                ap=[[P, FO], [1, P]],
            )

        @block.sync
        def _(sync):
            for t in range(NTILES):
                s, ci = t // NTSEQ, t % NTSEQ
                b = t % 2
                k0 = ci * FO
                if t >= 2:
                    sync.wait_ge(v_sem, t - 1)  # WAR: tile t-2 read its inputs
                src_a = AP(
                    tensor=xp_ext[:].tensor,
                    offset=s * TPC + k0 * P,
                    ap=[[P, FT], [1, W2]],
                )
                src_b = AP(
                    tensor=xp_ext[:].tensor,
                    offset=s * TPC + k0 * P + 1,
                    ap=[[P, FT], [1, W2]],
                )
                sync.dma_start(xa[b][:], src_a).then_inc(dma_s[b], 16)
                sync.dma_start(xb[b][:], src_b).then_inc(dma_s[b], 16)
                sync.dma_start(hh[b][:], hc_ext[s, k0 : k0 + FT, :]).then_inc(
                    dma_s[b], 16
                )
                if t == 0:
                    sync.dma_start(rrt[:], rr_ext[:]).then_inc(ramp_sem, 16)
                if t >= 1:
                    # partition-shift copy of V rows 1..FT for tile t-1
                    sync.wait_ge(v_sem, t)
                    sync.dma_start(vs[:], vt[1:FT, 0:P]).then_inc(vs_sem, 16)
                if t >= 2:
                    # store y of tile t-2 (ya available early; avoids blocking
                    # the next tile's input DMAs behind tile t-1's compute)
                    sync.wait_ge(ya_sem, t - 1)
                    sync.dma_start(ydst(t - 2), yt[(t - 2) % 2][:]).then_inc(
                        out_s[(t - 2) % 2], 16
                    )
            # tail: last tile's shift + remaining stores
            tl = NTILES - 1
            sync.wait_ge(v_sem, NTILES)
            sync.dma_start(vs[:], vt[1:FT, 0:P]).then_inc(vs_sem, 16)
            sync.wait_ge(ya_sem, NTILES - 1)
            sync.dma_start(ydst(tl - 1), yt[(tl - 1) % 2][:]).then_inc(
                out_s[(tl - 1) % 2], 16
            )
            sync.wait_ge(ya_sem, NTILES)
            sync.dma_start(ydst(tl), yt[tl % 2][:]).then_inc(out_s[tl % 2], 16)
            sync.wait_ge(out_s[tl % 2], 16 * (tl // 2 + 1))
            sync.wait_ge(out_s[1 - tl % 2], 16 * ((tl - 1) // 2 + 1))
            sync.sem_inc(cast_sem, 1)
            # after the gpsimd cast pass completes, kernel may end
            sync.wait_ge(cast_sem, 17)

        def src_for(buf_pair, b, j):
            # slice of the extended window for tap j, 4B-aligned via the
            # one-element-shifted copy when the natural offset is odd
            off = D - 1 - j
            if off % 2 == 0:
                return buf_pair[0][b][:, off : off + 2 * P]
            return buf_pair[1][b][:, off - 1 : off - 1 + 2 * P]

        @block.vector
        def _(vector):
            def conv(t):
                b = t % 2
                accs = [acc0, acc1]
                vector.wait_ge(dma_s[b], 48 * (t // 2 + 1))
                vector.tensor_scalar_mul(acc0[:], src_for((xa, xb), b, 0), hh[b][:, 0:1])
                cur = 0
                for j in range(1, N_DVE):
                    nxt = 1 - cur
                    vector.scalar_tensor_tensor(
                        out=accs[nxt][:],
                        in0=src_for((xa, xb), b, j),
                        scalar=hh[b][:, j : j + 1],
                        in1=accs[cur][:],
                        op0=mybir.AluOpType.mult,
                        op1=mybir.AluOpType.add,
                    )
                    cur = nxt
                # fold in the ACT-engine products: narrow pre-level folds the
                # slots beyond 16 onto the front (no zero padding needed), then
                # an in-place halving tree over the remaining 16 slots
                vector.wait_ge(act_sem, t + 1)
                if N_ACT > 16:
                    extra = N_ACT - 16
                    vector.tensor_tensor(
                        out=prb[b][:, 0 : extra * 2 * P],
                        in0=prb[b][:, 0 : extra * 2 * P],
                        in1=prb[b][:, 16 * 2 * P : N_ACT * 2 * P],
                        op=mybir.AluOpType.add,
                    )
                    width = 16 * 2 * P
                else:
                    width = NSLOT * 2 * P
                while width > 2 * P:
                    half = width // 2
                    vector.tensor_tensor(
                        out=prb[b][:, 0:half],
                        in0=prb[b][:, 0:half],
                        in1=prb[b][:, half:width],
                        op=mybir.AluOpType.add,
                    )
                    width = half
                nxt = 1 - cur
                vector.tensor_tensor(
                    out=accs[nxt][:],
                    in0=accs[cur][:],
                    in1=prb[b][:, 0 : 2 * P],
                    op=mybir.AluOpType.add,
                )
                cur = nxt
                return accs[cur]

            if N_ACT <= 16:
                # one-time zeroing of padding slots for the pure halving tree
                for pp in range(2):
                    vector.memset(prb[pp][:, N_ACT * 2 * P : NSLOT * 2 * P], 0.0)
            for t in range(NTILES):
                fin = conv(t)
                if t == 0:
                    vector.wait_ge(ramp_sem, 16)
                if t >= 1:
                    # combine tile t-1: y = V[0:FO, 80:160] + Vs
                    vector.wait_ge(vs_sem, 16 * t)
                    if t - 1 >= 2:
                        vector.wait_ge(out_s[(t - 1) % 2], 16 * ((t - 1) // 2))
                    vector.tensor_tensor(
                        out=yt[(t - 1) % 2][:],
                        in0=vt[0:FO, P : 2 * P],
                        in1=vs[:],
                        op=mybir.AluOpType.add,
                    ).then_inc(ya_sem, 1)
                # V_t = C_t * rr
                vector.tensor_tensor(
                    out=vt[:], in0=fin[:], in1=rrt[0:FT, :], op=mybir.AluOpType.mult
                ).then_inc(v_sem, 1)
            # tail combine for last tile
            tl = NTILES - 1
            vector.wait_ge(vs_sem, 16 * NTILES)
            vector.wait_ge(out_s[tl % 2], 16 * (tl // 2))
            vector.tensor_tensor(
                out=yt[tl % 2][:],
                in0=vt[0:FO, P : 2 * P],
                in1=vs[:],
                op=mybir.AluOpType.add,
            ).then_inc(ya_sem, 1)

        @block.scalar
        def _(scalar):
            for t in range(NTILES):
                b = t % 2
                scalar.wait_ge(dma_s[b], 48 * (t // 2 + 1))
                if t >= 2:
                    scalar.wait_ge(v_sem, t - 1)  # WAR on pr[b] scratch
                for idx, j in enumerate(range(N_DVE, D)):
                    inst = scalar.activation(
                        prb[b][:, idx * 2 * P : (idx + 1) * 2 * P],
                        src_for((xa, xb), b, j),
                        mybir.ActivationFunctionType.Copy,
                        scale=hh[b][:, j : j + 1],
                    )
                    if idx == N_ACT - 1:
                        inst.then_inc(act_sem, 1)

        @block.gpsimd
        def _(gpsimd):
            # final cast pass fp16 -> fp32 (SWDGE dtype-cast DMA)
            gpsimd.wait_ge(cast_sem, 1)
            gpsimd.dma_start(out_ext[:], ydram[:]).then_inc(cast_sem, 16)

    _nc_cache["nc"] = nc
    return nc


def _prep_core_inputs(x, h):
    x = np.ascontiguousarray(x, dtype=np.float32)
    h = np.ascontiguousarray(h, dtype=np.float32)
    xp = np.zeros((B, TPC), np.float16)
    xp[:, PAD : PAD + T] = x.astype(np.float16)
    hpad = np.ascontiguousarray(np.concatenate([h, h[:, -1:, :]], axis=1))  # (B,N+1,D) f32
    w1 = (np.arange(P, dtype=np.float32) / P).astype(np.float16)
    w0 = (1.0 - np.arange(P, dtype=np.float32) / P).astype(np.float16)
    rr = np.broadcast_to(
        np.concatenate([w1, w0])[None, :], (128, 2 * P)
    )
    rr = np.ascontiguousarray(rr)
    in_maps = []
    for c in range(NCORES):
        sl = slice(c * S, (c + 1) * S)
        in_maps.append({"xp": xp[sl], "hc": hpad[sl], "rr": rr})
    return in_maps


def kernel(x, h, **kw):
    nc = build_nc()
    in_maps = _prep_core_inputs(x, h)
    res = run_bass_kernel_spmd(nc, in_maps, core_ids=list(range(NCORES)), **kw)
    out = np.concatenate([res.results[c]["out"] for c in range(NCORES)], axis=0)
    return np.ascontiguousarray(out, dtype=np.float32)


def kernel_traced(x, h, **kw):
    nc = build_nc()
    in_maps = _prep_core_inputs(x, h)
    res = run_bass_kernel_spmd(
        nc, in_maps, core_ids=list(range(NCORES)), trace=True, **kw
    )
    out = np.concatenate([res.results[c]["out"] for c in range(NCORES)], axis=0)
    return np.ascontiguousarray(out, dtype=np.float32), res


# revision 41
# speedup vs baseline: 1.0372x; 1.0013x over previous
"""Time-varying FIR (AllZeroDigitalFilter) on 8 TRN2 NeuronCores.

fp16 "C-decomposition", dual-engine (Vector + Scalar):
  C_k[i'] = sum_j h_pad[k,j] * x[(k-1)P + i' - j],  i' in [0,160)
  (filter of frame k applied across frames k-1 and k)
  y[kP+i] = w0[i]*C_k[80+i] + w1[i]*C_{k+1}[i]
This halves op count vs the direct A/B blend: one FD=160 op per tap
instead of two FD=80 ops. Per 126-row tile, N_DVE taps run as fused
mult-add chains on the Vector engine (scalar_tensor_tensor); the other
D-N_DVE tap-products run concurrently on the Scalar engine
(activation Copy with per-partition scale) into a contiguous 32-slot
product buffer, folded into the accumulator on Vector by a narrow
pre-fold (slots 16..N_ACT onto the front) plus an in-place 16-slot
halving tree — all wide fp16 2x-mode adds, ~2.3us per tile. The cross-partition (+1) combine uses a
partition-shifted SBUF->SBUF DMA + one tensor add; output is staged in
an fp16 DRAM buffer and cast to fp32 with one SWDGE cast-DMA pass.
Precision (validated vs reference): ~7.9e-4 relative error.
Sharding: pure data parallel across batch, 2 sequences per core.

Sync design note: cumulative thresholds on a shared DMA semaphore are
unsound with >1 DMA in flight (per-SDMA-engine completion skew lets a
later tile's increments satisfy an earlier tile's threshold). Buffer-
parity semaphores make every threshold equal to the maximum possible
increment count at wait time, so a fired wait implies full completion.
"""

import sys

for p in ("/opt/trn_rl_repo", "/root/.axon_site/_ro/trn_rl_repo"):
    if p not in sys.path:
        sys.path.append(p)

import numpy as np
import concourse.bass as bass
import concourse.mybir as mybir
from concourse.ap import AP
from concourse.bass_utils import run_bass_kernel_spmd

B, T = 16, 80000
P, D = 80, 50  # frame period, taps
N = T // P  # 1000 frames
W2 = 2 * P + D - 1  # 209: extended window for the 160-wide C rows
NCORES = 8
S = B // NCORES  # sequences per core
FO = 125  # output frames per tile
FT = FO + 1  # C-rows per tile (tiles overlap by 1 row)
NTSEQ = N // FO  # 8 tiles per sequence
PAD = D - 1 + P  # front pad so C_k window starts at k*P: 129
TPC = N * P + W2 + 2  # padded x length (+2 slack for the odd-offset copy)

F16 = mybir.dt.float16
FP32 = mybir.dt.float32

N_DVE = 26  # taps computed on the Vector engine (fused mult-add chain)
# remaining D - N_DVE taps: products on the Scalar (ACT) engine, folded in
# with fp16 2x-mode tensor adds on DVE

_nc_cache = {}


def build_nc():
    if "nc" in _nc_cache:
        return _nc_cache["nc"]
    nc = bass.Bass()
    xp_ext = nc.declare_dram_parameter("xp", [S, TPC], F16, isOutput=False)
    hc_ext = nc.declare_dram_parameter("hc", [S, N + 1, D], FP32, isOutput=False)
    rr_ext = nc.declare_dram_parameter("rr", [128, 2 * P], F16, isOutput=False)
    out_ext = nc.declare_dram_parameter("out", [S, T], FP32, isOutput=True)
    ydram = nc.dram_tensor("ydram", [S, T], F16)

    NTILES = S * NTSEQ  # tile t -> seq s = t // NTSEQ, chunk ci = t % NTSEQ

    from contextlib import ExitStack

    with ExitStack() as _ctx:
        ec = _ctx.enter_context
        xa0 = ec(nc.sbuf_tensor([FT, W2], F16))
        xa1 = ec(nc.sbuf_tensor([FT, W2], F16))
        xb0 = ec(nc.sbuf_tensor([FT, W2], F16))
        xb1 = ec(nc.sbuf_tensor([FT, W2], F16))
        hh0 = ec(nc.sbuf_tensor([FT, D], FP32))
        hh1 = ec(nc.sbuf_tensor([FT, D], FP32))
        acc0 = ec(nc.sbuf_tensor([FT, 2 * P], F16))
        acc1 = ec(nc.sbuf_tensor([FT, 2 * P], F16))
        vt = ec(nc.sbuf_tensor([FT, 2 * P], F16))
        vs = ec(nc.sbuf_tensor([FO, P], F16))
        y0 = ec(nc.sbuf_tensor([FO, P], F16))
        y1 = ec(nc.sbuf_tensor([FO, P], F16))
        rrt = ec(nc.sbuf_tensor([128, 2 * P], F16))
        ramp_sem = ec(nc.semaphore("ramp_sem"))
        dma_e = ec(nc.semaphore("dma_e"))
        dma_o = ec(nc.semaphore("dma_o"))
        v_sem = ec(nc.semaphore("v_sem"))
        vs_sem = ec(nc.semaphore("vs_sem"))
        ya_sem = ec(nc.semaphore("ya_sem"))
        out_e = ec(nc.semaphore("out_e"))
        out_o = ec(nc.semaphore("out_o"))
        cast_sem = ec(nc.semaphore("cast_sem"))
        act_sem = ec(nc.semaphore("act_sem"))
        N_ACT = D - N_DVE
        NSLOT = 32  # padded to a power of two for the in-place halving tree
        assert N_ACT <= NSLOT
        prb0 = ec(nc.sbuf_tensor([FT, NSLOT * 2 * P], F16))
        prb1 = ec(nc.sbuf_tensor([FT, NSLOT * 2 * P], F16))
        prb = [prb0, prb1]
        block = ec(nc.Block())
        xa = [xa0, xa1]
        xb = [xb0, xb1]
        hh = [hh0, hh1]
        yt = [y0, y1]
        dma_s = [dma_e, dma_o]
        out_s = [out_e, out_o]

        def ydst(t):
            s, ci = t // NTSEQ, t % NTSEQ
            return AP(
                tensor=ydram[:].tensor,
                offset=s * T + ci * FO * P,
                ap=[[P, FO], [1, P]],
            )

        @block.sync
        def _(sync):
            for t in range(NTILES):
                s, ci = t // NTSEQ, t % NTSEQ
                b = t % 2
                k0 = ci * FO
                if t >= 2:
                    sync.wait_ge(v_sem, t - 1)  # WAR: tile t-2 read its inputs
                src_a = AP(
                    tensor=xp_ext[:].tensor,
                    offset=s * TPC + k0 * P,
                    ap=[[P, FT], [1, W2]],
                )
                src_b = AP(
                    tensor=xp_ext[:].tensor,
                    offset=s * TPC + k0 * P + 1,
                    ap=[[P, FT], [1, W2]],
                )
                sync.dma_start(xa[b][:], src_a).then_inc(dma_s[b], 16)
                sync.dma_start(xb[b][:], src_b).then_inc(dma_s[b], 16)
                sync.dma_start(hh[b][:], hc_ext[s, k0 : k0 + FT, :]).then_inc(
                    dma_s[b], 16
                )
                if t == 0:
                    sync.dma_start(rrt[:], rr_ext[:]).then_inc(ramp_sem, 16)
                if t >= 1:
                    # partition-shift copy of V rows 1..FT for tile t-1
                    sync.wait_ge(v_sem, t)
                    sync.dma_start(vs[:], vt[1:FT, 0:P]).then_inc(vs_sem, 16)
                if t >= 2:
                    # store y of tile t-2 (ya available early; avoids blocking
                    # the next tile's input DMAs behind tile t-1's compute)
                    sync.wait_ge(ya_sem, t - 1)
                    sync.dma_start(ydst(t - 2), yt[(t - 2) % 2][:]).then_inc(
                        out_s[(t - 2) % 2], 16
                    )
            # tail: last tile's shift + remaining stores
            tl = NTILES - 1
            sync.wait_ge(v_sem, NTILES)
            sync.dma_start(vs[:], vt[1:FT, 0:P]).then_inc(vs_sem, 16)
            sync.wait_ge(ya_sem, NTILES - 1)
            sync.dma_start(ydst(tl - 1), yt[(tl - 1) % 2][:]).then_inc(
                out_s[(tl - 1) % 2], 16
            )
            sync.wait_ge(ya_sem, NTILES)
            sync.dma_start(ydst(tl), yt[tl % 2][:]).then_inc(out_s[tl % 2], 16)
            sync.wait_ge(out_s[tl % 2], 16 * (tl // 2 + 1))
            sync.wait_ge(out_s[1 - tl % 2], 16 * ((tl - 1) // 2 + 1))
            sync.sem_inc(cast_sem, 1)
            # after the gpsimd cast pass completes, kernel may end
            sync.wait_ge(cast_sem, 17)

        def src_for(buf_pair, b, j):
            # slice of the extended window for tap j, 4B-aligned via the
            # one-element-shifted copy when the natural offset is odd
            off = D - 1 - j
            if off % 2 == 0:
                return buf_pair[0][b][:, off : off + 2 * P]
            return buf_pair[1][b][:, off - 1 : off - 1 + 2 * P]

        @block.vector
        def _(vector):
            def conv(t):
                b = t % 2
                accs = [acc0, acc1]
                vector.wait_ge(dma_s[b], 48 * (t // 2 + 1))
                vector.tensor_scalar_mul(acc0[:], src_for((xa, xb), b, 0), hh[b][:, 0:1])
                cur = 0
                for j in range(1, N_DVE):
                    nxt = 1 - cur
                    vector.scalar_tensor_tensor(
                        out=accs[nxt][:],
                        in0=src_for((xa, xb), b, j),
                        scalar=hh[b][:, j : j + 1],
                        in1=accs[cur][:],
                        op0=mybir.AluOpType.mult,
                        op1=mybir.AluOpType.add,
                    )
                    cur = nxt
                # fold in the ACT-engine products: narrow pre-level folds the
                # slots beyond 16 onto the front (no zero padding needed), then
                # an in-place halving tree over the remaining 16 slots
                vector.wait_ge(act_sem, t + 1)
                if N_ACT > 16:
                    extra = N_ACT - 16
                    vector.tensor_tensor(
                        out=prb[b][:, 0 : extra * 2 * P],
                        in0=prb[b][:, 0 : extra * 2 * P],
                        in1=prb[b][:, 16 * 2 * P : N_ACT * 2 * P],
                        op=mybir.AluOpType.add,
                    )
                    width = 16 * 2 * P
                else:
                    width = NSLOT * 2 * P
                while width > 2 * P:
                    half = width // 2
                    vector.tensor_tensor(
                        out=prb[b][:, 0:half],
                        in0=prb[b][:, 0:half],
                        in1=prb[b][:, half:width],
                        op=mybir.AluOpType.add,
                    )
                    width = half
                nxt = 1 - cur
                vector.tensor_tensor(
                    out=accs[nxt][:],
                    in0=accs[cur][:],
                    in1=prb[b][:, 0 : 2 * P],
                    op=mybir.AluOpType.add,
                )
                cur = nxt
                return accs[cur]

            if N_ACT <= 16:
                # one-time zeroing of padding slots for the pure halving tree
                for pp in range(2):
                    vector.memset(prb[pp][:, N_ACT * 2 * P : NSLOT * 2 * P], 0.0)
            for t in range(NTILES):
                fin = conv(t)
                if t == 0:
                    vector.wait_ge(ramp_sem, 16)
                if t >= 1:
                    # combine tile t-1: y = V[0:FO, 80:160] + Vs
                    vector.wait_ge(vs_sem, 16 * t)
                    if t - 1 >= 2:
                        vector.wait_ge(out_s[(t - 1) % 2], 16 * ((t - 1) // 2))
                    vector.tensor_tensor(
                        out=yt[(t - 1) % 2][:],
                        in0=vt[0:FO, P : 2 * P],
                        in1=vs[:],
                        op=mybir.AluOpType.add,
                    ).then_inc(ya_sem, 1)
                # V_t = C_t * rr
                vector.tensor_tensor(
                    out=vt[:], in0=fin[:], in1=rrt[0:FT, :], op=mybir.AluOpType.mult
                ).then_inc(v_sem, 1)
            # tail combine for last tile
            tl = NTILES - 1
            vector.wait_ge(vs_sem, 16 * NTILES)
            vector.wait_ge(out_s[tl % 2], 16 * (tl // 2))
            vector.tensor_tensor(
                out=yt[tl % 2][:],
                in0=vt[0:FO, P : 2 * P],
                in1=vs[:],
                op=mybir.AluOpType.add,
            ).then_inc(ya_sem, 1)

        @block.scalar
        def _(scalar):
            for t in range(NTILES):
                b = t % 2
                scalar.wait_ge(dma_s[b], 48 * (t // 2 + 1))
                if t >= 2:
                    scalar.wait_ge(v_sem, t - 1)  # WAR on pr[b] scratch
                for idx, j in enumerate(range(N_DVE, D)):
                    inst = scalar.activation(
                        prb[b][:, idx * 2 * P : (idx + 1) * 2 * P],
                        src_for((xa, xb), b, j),
                        mybir.ActivationFunctionType.Copy,
                        scale=hh[b][:, j : j + 1],
                    )
                    if idx == N_ACT - 1:
                        inst.then_inc(act_sem, 1)

        @block.gpsimd
        def _(gpsimd):
            # final cast pass fp16 -> fp32 (SWDGE dtype-cast DMA)
            gpsimd.wait_ge(cast_sem, 1)
            gpsimd.dma_start(out_ext[:], ydram[:]).then_inc(cast_sem, 16)

    _nc_cache["nc"] = nc
    return nc


def _prep_core_inputs(x, h):
    x = np.ascontiguousarray(x, dtype=np.float32)
    h = np.ascontiguousarray(h, dtype=np.float32)
    xp = np.zeros((B, TPC), np.float16)
    xp[:, PAD : PAD + T] = x.astype(np.float16)
    hpad = np.ascontiguousarray(np.concatenate([h, h[:, -1:, :]], axis=1))  # (B,N+1,D) f32
    w1 = (np.arange(P, dtype=np.float32) / P).astype(np.float16)
    w0 = (1.0 - np.arange(P, dtype=np.float32) / P).astype(np.float16)
    rr = np.broadcast_to(
        np.concatenate([w1, w0])[None, :], (128, 2 * P)
    )
    rr = np.ascontiguousarray(rr)
    in_maps = []
    for c in range(NCORES):
        sl = slice(c * S, (c + 1) * S)
        in_maps.append({"xp": xp[sl], "hc": hpad[sl], "rr": rr})
    return in_maps


def kernel(x, h, **kw):
    nc = build_nc()
    in_maps = _prep_core_inputs(x, h)
    res = run_bass_kernel_spmd(nc, in_maps, core_ids=list(range(NCORES)), **kw)
    out = np.concatenate([res.results[c]["out"] for c in range(NCORES)], axis=0)
    return np.ascontiguousarray(out, dtype=np.float32)


def kernel_traced(x, h, **kw):
    nc = build_nc()
    in_maps = _prep_core_inputs(x, h)
    res = run_bass_kernel_spmd(
        nc, in_maps, core_ids=list(range(NCORES)), trace=True, **kw
    )
    out = np.concatenate([res.results[c]["out"] for c in range(NCORES)], axis=0)
    return np.ascontiguousarray(out, dtype=np.float32), res


# revision 42
# speedup vs baseline: 1.2561x; 1.2110x over previous
"""Time-varying FIR (AllZeroDigitalFilter) on 8 TRN2 NeuronCores.

fp16 "C-decomposition", dual-engine (Vector + Scalar):
  C_k[i'] = sum_j h_pad[k,j] * x[(k-1)P + i' - j],  i' in [0,160)
  (filter of frame k applied across frames k-1 and k)
  y[kP+i] = w0[i]*C_k[80+i] + w1[i]*C_{k+1}[i]
This halves op count vs the direct A/B blend: one FD=160 op per tap
instead of two FD=80 ops. Per 126-row tile, N_DVE taps run as fused
mult-add chains on the Vector engine (scalar_tensor_tensor); the other
D-N_DVE tap-products run concurrently on the Scalar engine
(activation Copy with per-partition scale) into a contiguous 32-slot
product buffer, folded into the accumulator on Vector by a narrow
pre-fold (slots 16..N_ACT onto the front) plus an in-place 16-slot
halving tree — all wide fp16 2x-mode adds, ~2.3us per tile. The cross-partition (+1) combine uses a
partition-shifted SBUF->SBUF DMA + one tensor add; output is staged in
an fp16 DRAM buffer and cast to fp32 with one SWDGE cast-DMA pass.
Precision (validated vs reference): ~7.9e-4 relative error.
Sharding: pure data parallel across batch, 2 sequences per core.

Sync design note: cumulative thresholds on a shared DMA semaphore are
unsound with >1 DMA in flight (per-SDMA-engine completion skew lets a
later tile's increments satisfy an earlier tile's threshold). Buffer-
parity semaphores make every threshold equal to the maximum possible
increment count at wait time, so a fired wait implies full completion.
"""

import sys

for p in ("/opt/trn_rl_repo", "/root/.axon_site/_ro/trn_rl_repo"):
    if p not in sys.path:
        sys.path.append(p)

import numpy as np
import concourse.bass as bass
import concourse.mybir as mybir
from concourse.ap import AP
from concourse.bass_utils import run_bass_kernel_spmd

B, T = 16, 80000
P, D = 80, 50  # frame period, taps
N = T // P  # 1000 frames
W2 = 2 * P + D - 1  # 209: extended window for the 160-wide C rows
NCORES = 8
S = B // NCORES  # sequences per core
FO = 125  # output frames per tile
FT = FO + 1  # C-rows per tile (tiles overlap by 1 row)
NTSEQ = N // FO  # 8 tiles per sequence
PAD = D - 1 + P  # front pad so C_k window starts at k*P: 129
TPC = N * P + W2 + 2  # padded x length (+2 slack for the odd-offset copy)

F16 = mybir.dt.float16
FP32 = mybir.dt.float32

N_DVE = 26  # taps computed on the Vector engine (fused mult-add chain)
# remaining D - N_DVE taps: products on the Scalar (ACT) engine, folded in
# with fp16 2x-mode tensor adds on DVE

_nc_cache = {}


def build_nc():
    if "nc" in _nc_cache:
        return _nc_cache["nc"]
    nc = bass.Bass()
    xp_ext = nc.declare_dram_parameter("xp", [S, TPC], F16, isOutput=False)
    hc_ext = nc.declare_dram_parameter("hc", [S, N + 1, D], FP32, isOutput=False)
    rr_ext = nc.declare_dram_parameter("rr", [128, 2 * P], F16, isOutput=False)
    out_ext = nc.declare_dram_parameter("out", [S, T], FP32, isOutput=True)

    NTILES = S * NTSEQ  # tile t -> seq s = t // NTSEQ, chunk ci = t % NTSEQ

    from contextlib import ExitStack

    with ExitStack() as _ctx:
        ec = _ctx.enter_context
        xa0 = ec(nc.sbuf_tensor([FT, W2], F16))
        xa1 = ec(nc.sbuf_tensor([FT, W2], F16))
        xb0 = ec(nc.sbuf_tensor([FT, W2], F16))
        xb1 = ec(nc.sbuf_tensor([FT, W2], F16))
        hh0 = ec(nc.sbuf_tensor([FT, D], FP32))
        hh1 = ec(nc.sbuf_tensor([FT, D], FP32))
        acc0 = ec(nc.sbuf_tensor([FT, 2 * P], F16))
        acc1 = ec(nc.sbuf_tensor([FT, 2 * P], F16))
        vt = ec(nc.sbuf_tensor([FT, 2 * P], F16))
        vs = ec(nc.sbuf_tensor([FO, P], F16))
        y0 = ec(nc.sbuf_tensor([FO, P], FP32))
        y1 = ec(nc.sbuf_tensor([FO, P], FP32))
        rrt = ec(nc.sbuf_tensor([128, 2 * P], F16))
        ramp_sem = ec(nc.semaphore("ramp_sem"))
        dma_e = ec(nc.semaphore("dma_e"))
        dma_o = ec(nc.semaphore("dma_o"))
        v_sem = ec(nc.semaphore("v_sem"))
        vs_sem = ec(nc.semaphore("vs_sem"))
        ya_sem = ec(nc.semaphore("ya_sem"))
        out_e = ec(nc.semaphore("out_e"))
        out_o = ec(nc.semaphore("out_o"))
        act_sem = ec(nc.semaphore("act_sem"))
        N_ACT = D - N_DVE
        NSLOT = 32  # padded to a power of two for the in-place halving tree
        assert N_ACT <= NSLOT
        prb0 = ec(nc.sbuf_tensor([FT, NSLOT * 2 * P], F16))
        prb1 = ec(nc.sbuf_tensor([FT, NSLOT * 2 * P], F16))
        prb = [prb0, prb1]
        block = ec(nc.Block())
        xa = [xa0, xa1]
        xb = [xb0, xb1]
        hh = [hh0, hh1]
        yt = [y0, y1]
        dma_s = [dma_e, dma_o]
        out_s = [out_e, out_o]

        def ydst(t):
            s, ci = t // NTSEQ, t % NTSEQ
            return AP(
                tensor=out_ext[:].tensor,
                offset=s * T + ci * FO * P,
                ap=[[P, FO], [1, P]],
            )

        @block.sync
        def _(sync):
            for t in range(NTILES):
                s, ci = t // NTSEQ, t % NTSEQ
                b = t % 2
                k0 = ci * FO
                if t >= 2:
                    sync.wait_ge(v_sem, t - 1)  # WAR: tile t-2 read its inputs
                src_a = AP(
                    tensor=xp_ext[:].tensor,
                    offset=s * TPC + k0 * P,
                    ap=[[P, FT], [1, W2]],
                )
                src_b = AP(
                    tensor=xp_ext[:].tensor,
                    offset=s * TPC + k0 * P + 1,
                    ap=[[P, FT], [1, W2]],
                )
                sync.dma_start(xa[b][:], src_a).then_inc(dma_s[b], 16)
                sync.dma_start(xb[b][:], src_b).then_inc(dma_s[b], 16)
                sync.dma_start(hh[b][:], hc_ext[s, k0 : k0 + FT, :]).then_inc(
                    dma_s[b], 16
                )
                if t == 0:
                    sync.dma_start(rrt[:], rr_ext[:]).then_inc(ramp_sem, 16)
                if t >= 1:
                    # partition-shift copy of V rows 1..FT for tile t-1
                    sync.wait_ge(v_sem, t)
                    sync.dma_start(vs[:], vt[1:FT, 0:P]).then_inc(vs_sem, 16)
                if t >= 2:
                    # store y of tile t-2 (ya available early; avoids blocking
                    # the next tile's input DMAs behind tile t-1's compute)
                    sync.wait_ge(ya_sem, t - 1)
                    sync.dma_start(ydst(t - 2), yt[(t - 2) % 2][:]).then_inc(
                        out_s[(t - 2) % 2], 16
                    )
            # tail: last tile's shift + remaining stores
            tl = NTILES - 1
            sync.wait_ge(v_sem, NTILES)
            sync.dma_start(vs[:], vt[1:FT, 0:P]).then_inc(vs_sem, 16)
            sync.wait_ge(ya_sem, NTILES - 1)
            sync.dma_start(ydst(tl - 1), yt[(tl - 1) % 2][:]).then_inc(
                out_s[(tl - 1) % 2], 16
            )
            sync.wait_ge(ya_sem, NTILES)
            sync.dma_start(ydst(tl), yt[tl % 2][:]).then_inc(out_s[tl % 2], 16)
            sync.wait_ge(out_s[tl % 2], 16 * (tl // 2 + 1))
            sync.wait_ge(out_s[1 - tl % 2], 16 * ((tl - 1) // 2 + 1))


        def src_for(buf_pair, b, j):
            # slice of the extended window for tap j, 4B-aligned via the
            # one-element-shifted copy when the natural offset is odd
            off = D - 1 - j
            if off % 2 == 0:
                return buf_pair[0][b][:, off : off + 2 * P]
            return buf_pair[1][b][:, off - 1 : off - 1 + 2 * P]

        @block.vector
        def _(vector):
            def conv(t):
                b = t % 2
                accs = [acc0, acc1]
                vector.wait_ge(dma_s[b], 48 * (t // 2 + 1))
                vector.tensor_scalar_mul(acc0[:], src_for((xa, xb), b, 0), hh[b][:, 0:1])
                cur = 0
                for j in range(1, N_DVE):
                    nxt = 1 - cur
                    vector.scalar_tensor_tensor(
                        out=accs[nxt][:],
                        in0=src_for((xa, xb), b, j),
                        scalar=hh[b][:, j : j + 1],
                        in1=accs[cur][:],
                        op0=mybir.AluOpType.mult,
                        op1=mybir.AluOpType.add,
                    )
                    cur = nxt
                # fold in the ACT-engine products: narrow pre-level folds the
                # slots beyond 16 onto the front (no zero padding needed), then
                # an in-place halving tree over the remaining 16 slots
                vector.wait_ge(act_sem, t + 1)
                if N_ACT > 16:
                    extra = N_ACT - 16
                    vector.tensor_tensor(
                        out=prb[b][:, 0 : extra * 2 * P],
                        in0=prb[b][:, 0 : extra * 2 * P],
                        in1=prb[b][:, 16 * 2 * P : N_ACT * 2 * P],
                        op=mybir.AluOpType.add,
                    )
                    width = 16 * 2 * P
                else:
                    width = NSLOT * 2 * P
                while width > 2 * P:
                    half = width // 2
                    vector.tensor_tensor(
                        out=prb[b][:, 0:half],
                        in0=prb[b][:, 0:half],
                        in1=prb[b][:, half:width],
                        op=mybir.AluOpType.add,
                    )
                    width = half
                nxt = 1 - cur
                vector.tensor_tensor(
                    out=accs[nxt][:],
                    in0=accs[cur][:],
                    in1=prb[b][:, 0 : 2 * P],
                    op=mybir.AluOpType.add,
                )
                cur = nxt
                return accs[cur]

            if N_ACT <= 16:
                # one-time zeroing of padding slots for the pure halving tree
                for pp in range(2):
                    vector.memset(prb[pp][:, N_ACT * 2 * P : NSLOT * 2 * P], 0.0)
            for t in range(NTILES):
                fin = conv(t)
                if t == 0:
                    vector.wait_ge(ramp_sem, 16)
                if t >= 1:
                    # combine tile t-1: y = V[0:FO, 80:160] + Vs
                    vector.wait_ge(vs_sem, 16 * t)
                    if t - 1 >= 2:
                        vector.wait_ge(out_s[(t - 1) % 2], 16 * ((t - 1) // 2))
                    vector.tensor_tensor(
                        out=yt[(t - 1) % 2][:],
                        in0=vt[0:FO, P : 2 * P],
                        in1=vs[:],
                        op=mybir.AluOpType.add,
                    ).then_inc(ya_sem, 1)
                # V_t = C_t * rr
                vector.tensor_tensor(
                    out=vt[:], in0=fin[:], in1=rrt[0:FT, :], op=mybir.AluOpType.mult
                ).then_inc(v_sem, 1)
            # tail combine for last tile
            tl = NTILES - 1
            vector.wait_ge(vs_sem, 16 * NTILES)
            vector.wait_ge(out_s[tl % 2], 16 * (tl // 2))
            vector.tensor_tensor(
                out=yt[tl % 2][:],
                in0=vt[0:FO, P : 2 * P],
                in1=vs[:],
                op=mybir.AluOpType.add,
            ).then_inc(ya_sem, 1)

        @block.scalar
        def _(scalar):
            for t in range(NTILES):
                b = t % 2
                scalar.wait_ge(dma_s[b], 48 * (t // 2 + 1))
                if t >= 2:
                    scalar.wait_ge(v_sem, t - 1)  # WAR on pr[b] scratch
                for idx, j in enumerate(range(N_DVE, D)):
                    inst = scalar.activation(
                        prb[b][:, idx * 2 * P : (idx + 1) * 2 * P],
                        src_for((xa, xb), b, j),
                        mybir.ActivationFunctionType.Copy,
                        scale=hh[b][:, j : j + 1],
                    )
                    if idx == N_ACT - 1:
                        inst.then_inc(act_sem, 1)


    _nc_cache["nc"] = nc
    return nc


def _prep_core_inputs(x, h):
    x = np.ascontiguousarray(x, dtype=np.float32)
    h = np.ascontiguousarray(h, dtype=np.float32)
    xp = np.zeros((B, TPC), np.float16)
    xp[:, PAD : PAD + T] = x.astype(np.float16)
    hpad = np.ascontiguousarray(np.concatenate([h, h[:, -1:, :]], axis=1))  # (B,N+1,D) f32
    w1 = (np.arange(P, dtype=np.float32) / P).astype(np.float16)
    w0 = (1.0 - np.arange(P, dtype=np.float32) / P).astype(np.float16)
    rr = np.broadcast_to(
        np.concatenate([w1, w0])[None, :], (128, 2 * P)
    )
    rr = np.ascontiguousarray(rr)
    in_maps = []
    for c in range(NCORES):
        sl = slice(c * S, (c + 1) * S)
        in_maps.append({"xp": xp[sl], "hc": hpad[sl], "rr": rr})
    return in_maps


def kernel(x, h, **kw):
    nc = build_nc()
    in_maps = _prep_core_inputs(x, h)
    res = run_bass_kernel_spmd(nc, in_maps, core_ids=list(range(NCORES)), **kw)
    out = np.concatenate([res.results[c]["out"] for c in range(NCORES)], axis=0)
    return np.ascontiguousarray(out, dtype=np.float32)


def kernel_traced(x, h, **kw):
    nc = build_nc()
    in_maps = _prep_core_inputs(x, h)
    res = run_bass_kernel_spmd(
        nc, in_maps, core_ids=list(range(NCORES)), trace=True, **kw
    )
    out = np.concatenate([res.results[c]["out"] for c in range(NCORES)], axis=0)
    return np.ascontiguousarray(out, dtype=np.float32), res
